# revision 24
# baseline (speedup 1.0000x reference)
"""GraphSAGE mean-aggregation kernel for one TRN2 chip (8 NeuronCores).

Reference computation (see problem):
    h    = feat @ w_neigh.T                      [N, 128]
    msg  = h[src]                                [E, 128]
    agg  = segment_sum(msg, dst, N)              [N, 128]
    deg  = segment_sum(ones, dst, N)
    out  = agg / max(deg, 1) + bias

Distribution (dst-sharded graph parallel):
  - Nodes sharded 8 x 12500. Core i owns dst rows [i*12500, (i+1)*12500).
  - Each core computes h for its own node shard (TensorE matmul, bf16) in 4
    row-slices; each slice is AllGather'd separately so bucket-0 gathers
    start while later slices are still in flight.  The h table is PERMUTED:
    slice q = concat over cores of their local rows [O[q], O[q+1]).
  - Edge messages are fetched with gpsimd.dma_gather (256B rows) from the
    per-slice tables (<=25000 rows each -> int16 indices).  The main loop is
    BUCKET-major: pass q drains every batch's bucket-q chunks, accumulating
    windows into an SBUF f32 accumulator (psum -> DVE add), so early passes
    fully overlap the remaining collectives.
  - The per-chunk one-hot (segment-sum matmul lhsT) is generated ON-CHIP by
    DVE: ohb[p, j, s] = (slot[p, j] == iota[s]), from a tiny slot stream.
  - Epilogue (pass 3) fuses mean (per-partition scale) and bias, then DMAs
    the batch out.

SPMD: all 8 cores run one instruction stream; the chunk schedule (C_wq
chunks per (window, bucket)) is the max over cores.  A 4-dim balancing
greedy assigns nodes to NW=99 windows so counts pack chunks of 128 almost
exactly (pad gather idx 0, pad one-hot slot -1 -> all-zero column).
"""

import sys

sys.path.insert(0, "/opt/trn_rl_repo")

import numpy as np

N_NODES = 100000
N_CORES = 8
SHARD = N_NODES // N_CORES  # 12500
D_IN = 256
D_OUT = 128
NT = (SHARD + 127) // 128  # 98 prologue tiles over h_loc rows
NW = 99  # dst aggregation windows (12672 slots; slack for 4-dim packing)
O = [0, 3125, 6250, 9375, 12500]  # h_loc row-slice boundaries (equal AG slices)
NQ = 4
WPB = 4  # windows per batch
NB = (NW + WPB - 1) // WPB  # 25 batches of 4 windows
SHARD_PAD = NW * 128  # 12672

_cache = {}


def _assign_windows(w4):
    """Assign 12500 nodes to NW windows (<=128 each), balancing the 4-dim
    per-bucket edge counts.  w4: [SHARD, 4] edge counts per node per bucket."""
    tot = w4.sum(axis=1)
    order = np.argsort(-tot, kind="stable")
    loads = np.zeros((NW, NQ))
    cnt = np.zeros(NW, np.int64)
    cap = np.full(NW, 128, np.int64)
    CHUNK_CAP = 512.0  # 4 chunks of 128: stay under to keep C_wq == 4
    T = 500.0
    assign = np.empty(SHARD, np.int64)
    for n in order:
        x = w4[n]
        cand = loads + x
        over_cap = np.maximum(cand - CHUNK_CAP, 0.0)
        over_t = np.maximum(cand - T, 0.0)
        score = 1e6 * over_cap.sum(axis=1) + (over_t * over_t).sum(axis=1) + 1e-4 * cand.sum(axis=1)
        score[cnt >= cap] = np.inf
        w = int(np.argmin(score))
        assign[n] = w
        loads[w] += x
        cnt[w] += 1
    return assign


def _make_runs(C_wq):
    """Bucket-major chunk schedule: for q, for batch: the batch's windows'
    bucket-q chunks, consecutively."""
    runs = []
    ch = 0
    gs = 0
    for q in range(NQ):
        for b in range(NB):
            ws = list(range(b * WPB, min((b + 1) * WPB, NW)))
            wl = []
            off = 0
            for w in ws:
                c = int(C_wq[w, q])
                if c > 0:
                    wl.append((w, off, c))
                off += c
            runs.append(
                {"q": q, "b": b, "ws": ws, "ch_base": ch, "nch": off, "wl": wl, "gs_off": gs}
            )
            ch += off
            gs += off * 9
    return runs, ch


def _preprocess(feat, w_neigh, bias, src, dst):
    src = np.asarray(src).astype(np.int64)
    dst = np.asarray(dst).astype(np.int64)
    feat = np.asarray(feat, np.float32)

    Ob = np.asarray(O)
    sz = Ob[1:] - Ob[:-1]  # slice sizes per core
    # table position of global node s (core i, local m): bucket qq = slice of m
    # bucket-local index = i*sz[qq] + (m - O[qq])
    s_core = src // SHARD
    s_m = src % SHARD
    s_q = np.searchsorted(Ob[1:], s_m, side="right")  # bucket of each edge src
    s_loc = s_core * sz[s_q] + (s_m - Ob[s_q])  # idx within bucket view

    core_of = dst // SHARD
    counts = np.zeros((N_CORES, NW * NQ), np.int64)
    per_core_raw = []
    perms = []  # per core: node -> padded out row (w*128 + slot)
    for i in range(N_CORES):
        m = core_of == i
        ed = dst[m] - i * SHARD
        eq = s_q[m]
        el = s_loc[m]
        # 4-dim balanced window assignment
        w4 = np.zeros((SHARD, NQ), np.int64)
        np.add.at(w4, (ed, eq), 1)
        assign_w = _assign_windows(w4)
        # slot within window
        order_by_w = np.argsort(assign_w, kind="stable")
        slot_of = np.empty(SHARD, np.int64)
        wcnt = np.bincount(assign_w, minlength=NW)
        woff = np.concatenate([[0], np.cumsum(wcnt)])
        slot_of[order_by_w] = np.arange(SHARD) - np.repeat(woff[:-1], wcnt)
        perms.append(assign_w * 128 + slot_of)

        w = assign_w[ed]
        sl_v = slot_of[ed]
        key = w * NQ + eq
        counts[i] = np.bincount(key, minlength=NW * NQ)
        order = np.argsort(key, kind="stable")
        per_core_raw.append((el[order], sl_v[order], np.concatenate([[0], np.cumsum(counts[i])])))

    # shared schedule: chunks per (window, bucket)
    cmax = counts.max(axis=0).reshape(NW, NQ)
    C_wq = (cmax + 127) // 128  # [NW, NQ]
    runs, totch = _make_runs(C_wq)

    # per-core edge streams in schedule order
    import ml_dtypes

    in_maps = []
    wT = np.ascontiguousarray(w_neigh.T).astype(np.float32)  # [256, 128]
    bias_bc = np.tile(np.asarray(bias, np.float32)[None, :], (128, 1))
    iota_bc = np.ascontiguousarray(
        np.broadcast_to(np.arange(128, dtype=np.float32), (128, 128))
    ).astype(ml_dtypes.bfloat16)

    for i in range(N_CORES):
        es, sl, goff = per_core_raw[i]
        gidx_stream = np.zeros(totch * 128, np.int16)
        slot_stream = np.full(totch * 128, -1.0, np.float32)
        for r in runs:
            for (w, woff, c) in r["wl"]:
                g = w * NQ + r["q"]
                n = goff[g + 1] - goff[g]
                base = (r["ch_base"] + woff) * 128
                gidx_stream[base : base + n] = es[goff[g] : goff[g + 1]]
                slot_stream[base : base + n] = sl[goff[g] : goff[g + 1]]
        # combined stream, run-major: per run [idx cols nch*8 | slot cols nch]
        # idx wrapped [16, nch*8] tiled to 128 partitions; slot bf16 bits,
        # slot_bf[p, P] = slot of edge P*128+p.
        slot_bf = (
            np.ascontiguousarray(slot_stream.reshape(totch, 128).T)
            .astype(ml_dtypes.bfloat16)
            .view(np.int16)
        )  # [128, totch]
        blocks = []
        for r in runs:
            if r["nch"] == 0:
                continue
            beg = r["ch_base"] * 128
            seg = gidx_stream[beg : beg + r["nch"] * 128]
            blocks.append(np.tile(seg.reshape(-1, 16).T, (8, 1)))  # [128, nch*8]
            blocks.append(slot_bf[:, r["ch_base"] : r["ch_base"] + r["nch"]])
        gw = np.hstack(blocks)  # [128, totch*9]

        ed_full = dst[core_of == i] - i * SHARD
        deg = np.bincount(ed_full, minlength=SHARD)
        recip = np.ones(SHARD_PAD, np.float32)
        recip[perms[i]] = (1.0 / np.maximum(deg, 1)).astype(np.float32)
        recip = np.ascontiguousarray(recip.reshape(NW, 128).T)  # [128, NW]

        featT = np.ascontiguousarray(feat[i * SHARD : (i + 1) * SHARD].T).astype(
            ml_dtypes.bfloat16
        )  # [256, 12500] bf16

        in_maps.append(
            {
                "featT": featT,
                "wT": wT,
                "bias_bc": bias_bc,
                "iota_bc": iota_bc,
                "recip": recip,
                "gidx": np.ascontiguousarray(gw),
            }
        )
    return in_maps, {"runs": runs, "totch": totch, "C_wq": C_wq, "perms": perms}


def _build(sched):
    import os

    from concourse import bacc, mybir, tile

    max_runs = int(os.environ.get("K_MAX_RUNS", "9999"))
    skip_ag = os.environ.get("K_SKIP_AG", "0") == "1"
    skip_gather = os.environ.get("K_SKIP_GATHER", "0") == "1"
    oh_dtype_name = os.environ.get("K_OH_DT", "float8e4")
    no_packet = os.environ.get("K_NO_PACKET", "0") == "1"

    runs = sched["runs"]
    totch = sched["totch"]
    f32 = mybir.dt.float32
    bf16 = mybir.dt.bfloat16
    i16 = mybir.dt.int16
    oh_dt = getattr(mybir.dt, oh_dtype_name)

    nc = bacc.Bacc(num_devices=N_CORES, num_swdge_queues=4, dynamic_dma_scratch_size=32768)
    featT = nc.dram_tensor("featT", [D_IN, SHARD], bf16, kind="ExternalInput")
    wT = nc.dram_tensor("wT", [D_IN, D_OUT], f32, kind="ExternalInput")
    bias_bc = nc.dram_tensor("bias_bc", [128, D_OUT], f32, kind="ExternalInput")
    iota_in = nc.dram_tensor("iota_bc", [128, 128], bf16, kind="ExternalInput")
    recip_in = nc.dram_tensor("recip", [128, NW], f32, kind="ExternalInput")
    gidx_in = nc.dram_tensor("gidx", [128, totch * 9], i16, kind="ExternalInput")
    out = nc.dram_tensor("out", [SHARD_PAD, D_OUT], f32, kind="ExternalOutput")

    # AG slice q fires after the prologue tile containing row O[q+1]-1
    ag_after_tile = {(O[q + 1] + 127) // 128 - 1: q for q in range(NQ)}

    with tile.TileContext(nc) as tc:
        with (
            tc.tile_pool(name="dram", bufs=1, space="DRAM") as dram,
            tc.tile_pool(name="const", bufs=1) as constp,
            tc.tile_pool(name="ft", bufs=1) as ftp,
            tc.tile_pool(name="accp", bufs=1) as accp,
            tc.tile_pool(name="sb", bufs=2) as sb,
            tc.tile_pool(name="hbp", bufs=4) as hbp,
            tc.tile_pool(name="psA", bufs=2, space="PSUM") as psA,
            tc.tile_pool(name="psB", bufs=6, space="PSUM") as psB,
        ):
            h_loc = [
                dram.tile([O[q + 1] - O[q], D_OUT], bf16, name=f"h_loc{q}")
                for q in range(NQ)
            ]
            h_slice = [
                dram.tile(
                    [8 * (O[q + 1] - O[q]), D_OUT],
                    bf16,
                    addr_space="Shared",
                    name=f"h_slice{q}",
                )
                for q in range(NQ)
            ]

            # constants
            wt = constp.tile([128, 2, D_OUT], bf16)
            nc.gpsimd.dma_start(out=wt[:], in_=wT[:, :].rearrange("(a k) n -> k a n", k=128))
            biast = constp.tile([128, D_OUT], f32)
            nc.sync.dma_start(out=biast[:], in_=bias_bc[:, :])
            iotab = constp.tile([128, 128], bf16)
            nc.sync.dma_start(out=iotab[:], in_=iota_in[:, :])
            recip = constp.tile([128, NW], f32)
            nc.sync.dma_start(out=recip[:], in_=recip_in[:, :])

            acc = accp.tile([128, NW, D_OUT], f32)
            nc.vector.memset(acc[:], 0.0)

            # ---- prologue: h = (feat @ w.T) bf16, sliced; AG per slice ----
            # featT slice tiles are padded out to 128-col tile boundaries so
            # every 128-row h tile's lhsT comes from ONE slice tile (psum
            # outputs must start at partition 0); only the h_loc store DMA
            # splits at slice boundaries.
            # featT slice loads are issued just-in-time at each slice start
            # (sync + scalar queues) so slice-0 h_loc writes are not queued
            # behind 24us of remaining featT loads on the serial Sync queue.
            ft = []
            ft_rng = []
            for q in range(NQ):
                plo = (O[q] // 128) * 128
                phi = min(((O[q + 1] + 127) // 128) * 128, SHARD)
                f = ftp.tile([128, 2, phi - plo], bf16, name=f"ft{q}")
                ft.append(f)
                ft_rng.append((plo, phi))
            Onp = np.asarray(O)
            loaded = set()
            for t in range(NT):
                lo = t * 128
                cnt = min(128, SHARD - lo)
                qt = int(np.searchsorted(Onp[1:], lo, side="right"))  # slice of tile
                if qt not in loaded:
                    loaded.add(qt)
                    plo, phi = ft_rng[qt]
                    nc.sync.dma_start(out=ft[qt][:, 0, :], in_=featT[0:128, plo:phi])
                    nc.scalar.dma_start(out=ft[qt][:, 1, :], in_=featT[128:256, plo:phi])
                ph = psA.tile([128, D_OUT], f32, space="PSUM")
                nc.tensor.matmul(
                    ph[:cnt, :],
                    lhsT=ft[qt][:, 0, lo - ft_rng[qt][0] : lo - ft_rng[qt][0] + cnt],
                    rhs=wt[:, 0, :],
                    start=True,
                    stop=False,
                )
                nc.tensor.matmul(
                    ph[:cnt, :],
                    lhsT=ft[qt][:, 1, lo - ft_rng[qt][0] : lo - ft_rng[qt][0] + cnt],
                    rhs=wt[:, 1, :],
                    start=False,
                    stop=True,
                )
                hb = hbp.tile([128, D_OUT], bf16)
                nc.scalar.activation(hb[:cnt, :], ph[:cnt, :], mybir.ActivationFunctionType.Copy)
                for q in range(NQ):
                    r0, r1 = max(lo, O[q]), min(lo + cnt, O[q + 1])
                    if r0 < r1:
                        nc.sync.dma_start(
                            out=h_loc[q][r0 - O[q] : r1 - O[q], :],
                            in_=hb[r0 - lo : r1 - lo, :],
                        )
                q = ag_after_tile.get(t)
                if q is not None:
                    if skip_ag:
                        nc.sync.dma_start(
                            out=h_slice[q][0 : O[q + 1] - O[q], :], in_=h_loc[q][:]
                        )
                    else:
                        nc.gpsimd.collective_compute(
                            "AllGather",
                            mybir.AluOpType.bypass,
                            replica_groups=[list(range(N_CORES))],
                            ins=[h_loc[q][:].opt()],
                            outs=[h_slice[q][:].opt()],
                        )

            # ---- main loop: bucket-major passes ----
            # one combined idx+slot stream load per pass (4 big DMAs on the
            # otherwise-idle Scalar HWDGE queue) -- many tiny per-run loads
            # head-of-line block the serial Sync queue.
            qrr = [0]
            pass_tiles = {}
            for q in range(NQ):
                pruns = [r for r in runs if r["q"] == q and r["nch"] > 0]
                c0 = pruns[0]["gs_off"]
                c1 = pruns[-1]["gs_off"] + pruns[-1]["nch"] * 9
                gsp = sb.tile([128, c1 - c0], i16, tag="gsp", bufs=2, name=f"gsp{q}")
                nc.scalar.dma_start(out=gsp[:], in_=gidx_in[:, c0:c1])
                pass_tiles[q] = (gsp, c0)
            for r in runs[:max_runs]:
                nch = r["nch"]
                if nch == 0:
                    continue
                q = r["q"]
                gsp, pc0 = pass_tiles[q]
                ro = r["gs_off"] - pc0
                gi = gsp[:, ro : ro + nch * 8]
                slt = gsp[:, ro + nch * 8 : ro + nch * 9].bitcast(bf16)
                ohb = sb.tile([128, nch, 128], oh_dt, tag="ohb", bufs=3)
                nc.vector.tensor_tensor(
                    out=ohb[:],
                    in0=slt[:, :, None].broadcast_to([128, nch, 128]),
                    in1=iotab[:, None, :].broadcast_to([128, nch, 128]),
                    op=mybir.AluOpType.is_equal,
                )
                msg = sb.tile([128, nch, D_OUT], bf16, tag="msg", bufs=6)
                if not skip_gather:
                    if no_packet:
                        # one unpacketed call per run: trades per-descriptor
                        # drain efficiency for ~half the Pool-engine call
                        # overhead (994ns fixed per SWDGE op)
                        nc.gpsimd.dma_gather(
                            msg[:, :, :],
                            h_slice[q][:],
                            gi[:, : nch * 8],
                            num_idxs=nch * 128,
                            num_idxs_reg=nch * 128,
                            elem_size=D_OUT,
                            queue_num=qrr[0] % 4,
                            single_packet=False,
                        )
                        qrr[0] += 1
                    else:
                        for s0 in range(0, nch, 8):
                            sc = min(8, nch - s0)
                            nc.gpsimd.dma_gather(
                                msg[:, s0 : s0 + sc, :],
                                h_slice[q][:],
                                gi[:, s0 * 8 : (s0 + sc) * 8],
                                num_idxs=sc * 128,
                                num_idxs_reg=sc * 128,
                                elem_size=D_OUT,
                                queue_num=qrr[0] % 4,
                                single_packet=True,
                            )
                            qrr[0] += 1
                ot = None
                if q == NQ - 1:
                    ot = sb.tile([128, len(r["ws"]), D_OUT], f32, tag="ot")
                for wi, w in enumerate(r["ws"]):
                    runs_w = [(woff, c) for (ww, woff, c) in r["wl"] if ww == w]
                    total_c = sum(c for _, c in runs_w)
                    if total_c == 0:
                        continue
                    pw = psB.tile([128, D_OUT], f32, space="PSUM", tag="pw")
                    done = 0
                    for (base, c) in runs_w:
                        for j in range(c):
                            nc.tensor.matmul(
                                pw[:, :],
                                lhsT=ohb[:, base + j, :],
                                rhs=msg[:, base + j, :],
                                start=(done == 0),
                                stop=(done == total_c - 1),
                            )
                            done += 1
                    nc.vector.tensor_add(acc[:, w, :], acc[:, w, :], pw[:, :])
                    if q == NQ - 1:
                        nc.scalar.activation(
                            ot[:, wi, :], acc[:, w, :], mybir.ActivationFunctionType.Copy,
                            scale=recip[:, w : w + 1],
                        )
                        nc.vector.tensor_add(ot[:, wi, :], ot[:, wi, :], biast[:])
                if q == NQ - 1:
                    nw_b = len(r["ws"])
                    w0 = r["ws"][0]
                    nc.sync.dma_start(
                        out=out[w0 * 128 : w0 * 128 + nw_b * 128, :].rearrange(
                            "(c p) f -> p c f", p=128
                        ),
                        in_=ot[:],
                    )

    nc.finalize()
    return nc


def _run(inputs, trace=False):
    from concourse.bass_utils import run_bass_kernel_spmd

    key = "k"
    in_maps, sched = _preprocess(
        inputs["feat"], inputs["w_neigh"], inputs["bias"], inputs["src"], inputs["dst"]
    )
    if key not in _cache:
        _cache[key] = _build(sched)
    nc = _cache[key]
    res = run_bass_kernel_spmd(nc, in_maps, core_ids=list(range(N_CORES)), trace=trace)
    outs = [res.results[i]["out"][sched["perms"][i]] for i in range(N_CORES)]
    full = np.concatenate(outs, axis=0)
    return full, res


def kernel(**inputs):
    full, _ = _run(inputs, trace=False)
    return full


# revision 31
# speedup vs baseline: 2.8295x; 2.8295x over previous
"""GraphSAGE mean-aggregation kernel for one TRN2 chip (8 NeuronCores).

Reference computation (see problem):
    h    = feat @ w_neigh.T                      [N, 128]
    msg  = h[src]                                [E, 128]
    agg  = segment_sum(msg, dst, N)              [N, 128]
    deg  = segment_sum(ones, dst, N)
    out  = agg / max(deg, 1) + bias

Distribution (dst-sharded graph parallel):
  - Nodes sharded 8 x 12500. Core i owns dst rows [i*12500, (i+1)*12500).
  - Each core computes h for its own node shard (TensorE matmul, bf16) in 4
    row-slices; each slice is AllGather'd separately so bucket-0 gathers
    start while later slices are still in flight.  The h table is PERMUTED:
    slice q = concat over cores of their local rows [O[q], O[q+1]).
  - Edge messages are fetched with gpsimd.dma_gather (256B rows) from the
    per-slice tables (<=25000 rows each -> int16 indices).  The main loop is
    BUCKET-major: pass q drains every batch's bucket-q chunks, accumulating
    windows into an SBUF f32 accumulator (psum -> DVE add), so early passes
    fully overlap the remaining collectives.
  - The per-chunk one-hot (segment-sum matmul lhsT) is generated ON-CHIP by
    DVE: ohb[p, j, s] = (slot[p, j] == iota[s]), from a tiny slot stream.
  - Epilogue (pass 3) fuses mean (per-partition scale) and bias, then DMAs
    the batch out.

SPMD: all 8 cores run one instruction stream; the chunk schedule (C_wq
chunks per (window, bucket)) is the max over cores.  A 4-dim balancing
greedy assigns nodes to NW=99 windows so counts pack chunks of 128 almost
exactly (pad gather idx 0, pad one-hot slot -1 -> all-zero column).
"""

import sys

sys.path.insert(0, "/opt/trn_rl_repo")

import numpy as np

N_NODES = 100000
N_CORES = 8
SHARD = N_NODES // N_CORES  # 12500
D_IN = 256
D_OUT = 128
NT = (SHARD + 127) // 128  # 98 prologue tiles over h_loc rows
NW = 99  # dst aggregation windows (12672 slots; slack for 4-dim packing)
O = [0, 3125, 6250, 9375, 12500]  # h_loc row-slice boundaries (equal AG slices)
NQ = 4
WPB = 4  # windows per batch
NB = (NW + WPB - 1) // WPB  # 25 batches of 4 windows
SHARD_PAD = NW * 128  # 12672

_cache = {}


def _assign_windows(w4):
    """Assign 12500 nodes to NW windows (<=128 each), balancing the 4-dim
    per-bucket edge counts.  w4: [SHARD, 4] edge counts per node per bucket."""
    tot = w4.sum(axis=1)
    order = np.argsort(-tot, kind="stable")
    loads = np.zeros((NW, NQ))
    cnt = np.zeros(NW, np.int64)
    cap = np.full(NW, 128, np.int64)
    CHUNK_CAP = 512.0  # 4 chunks of 128: stay under to keep C_wq == 4
    T = 500.0
    assign = np.empty(SHARD, np.int64)
    for n in order:
        x = w4[n]
        cand = loads + x
        over_cap = np.maximum(cand - CHUNK_CAP, 0.0)
        over_t = np.maximum(cand - T, 0.0)
        score = 1e6 * over_cap.sum(axis=1) + (over_t * over_t).sum(axis=1) + 1e-4 * cand.sum(axis=1)
        score[cnt >= cap] = np.inf
        w = int(np.argmin(score))
        assign[n] = w
        loads[w] += x
        cnt[w] += 1
    return assign


def _make_runs(C_wq):
    """Bucket-major chunk schedule: for q, for batch: the batch's windows'
    bucket-q chunks, consecutively."""
    runs = []
    ch = 0
    gs = 0
    for q in range(NQ):
        for b in range(NB):
            ws = list(range(b * WPB, min((b + 1) * WPB, NW)))
            wl = []
            off = 0
            for w in ws:
                c = int(C_wq[w, q])
                if c > 0:
                    wl.append((w, off, c))
                off += c
            runs.append(
                {"q": q, "b": b, "ws": ws, "ch_base": ch, "nch": off, "wl": wl, "gs_off": gs}
            )
            ch += off
            gs += off * 9
    return runs, ch


def _preprocess(feat, w_neigh, bias, src, dst):
    src = np.asarray(src).astype(np.int64)
    dst = np.asarray(dst).astype(np.int64)
    feat = np.asarray(feat, np.float32)

    Ob = np.asarray(O)
    sz = Ob[1:] - Ob[:-1]  # slice sizes per core
    # table position of global node s (core i, local m): bucket qq = slice of m
    # bucket-local index = i*sz[qq] + (m - O[qq])
    s_core = src // SHARD
    s_m = src % SHARD
    s_q = np.searchsorted(Ob[1:], s_m, side="right")  # bucket of each edge src
    s_loc = s_core * sz[s_q] + (s_m - Ob[s_q])  # idx within bucket view

    core_of = dst // SHARD
    counts = np.zeros((N_CORES, NW * NQ), np.int64)
    per_core_raw = []
    perms = []  # per core: node -> padded out row (w*128 + slot)
    for i in range(N_CORES):
        m = core_of == i
        ed = dst[m] - i * SHARD
        eq = s_q[m]
        el = s_loc[m]
        # 4-dim balanced window assignment
        w4 = np.zeros((SHARD, NQ), np.int64)
        np.add.at(w4, (ed, eq), 1)
        assign_w = _assign_windows(w4)
        # slot within window
        order_by_w = np.argsort(assign_w, kind="stable")
        slot_of = np.empty(SHARD, np.int64)
        wcnt = np.bincount(assign_w, minlength=NW)
        woff = np.concatenate([[0], np.cumsum(wcnt)])
        slot_of[order_by_w] = np.arange(SHARD) - np.repeat(woff[:-1], wcnt)
        perms.append(assign_w * 128 + slot_of)

        w = assign_w[ed]
        sl_v = slot_of[ed]
        key = w * NQ + eq
        counts[i] = np.bincount(key, minlength=NW * NQ)
        order = np.argsort(key, kind="stable")
        per_core_raw.append((el[order], sl_v[order], np.concatenate([[0], np.cumsum(counts[i])])))

    # shared schedule: chunks per (window, bucket)
    cmax = counts.max(axis=0).reshape(NW, NQ)
    C_wq = (cmax + 127) // 128  # [NW, NQ]
    runs, totch = _make_runs(C_wq)

    # per-core edge streams in schedule order
    import ml_dtypes

    in_maps = []
    wT = np.ascontiguousarray(w_neigh.T).astype(np.float32)  # [256, 128]
    bias_bc = np.tile(np.asarray(bias, np.float32)[None, :], (128, 1))
    iota_bc = np.ascontiguousarray(
        np.broadcast_to(np.arange(128, dtype=np.float32), (128, 128))
    ).astype(ml_dtypes.bfloat16)

    for i in range(N_CORES):
        es, sl, goff = per_core_raw[i]
        gidx_stream = np.zeros(totch * 128, np.int16)
        slot_stream = np.full(totch * 128, -1.0, np.float32)
        for r in runs:
            for (w, woff, c) in r["wl"]:
                g = w * NQ + r["q"]
                n = goff[g + 1] - goff[g]
                base = (r["ch_base"] + woff) * 128
                gidx_stream[base : base + n] = es[goff[g] : goff[g + 1]]
                slot_stream[base : base + n] = sl[goff[g] : goff[g + 1]]
        # combined stream, run-major: per run [idx cols nch*8 | slot cols nch]
        # idx wrapped [16, nch*8] tiled to 128 partitions; slot bf16 bits,
        # slot_bf[p, P] = slot of edge P*128+p.
        slot_bf = (
            np.ascontiguousarray(slot_stream.reshape(totch, 128).T)
            .astype(ml_dtypes.bfloat16)
            .view(np.int16)
        )  # [128, totch]
        blocks = []
        for r in runs:
            if r["nch"] == 0:
                continue
            beg = r["ch_base"] * 128
            seg = gidx_stream[beg : beg + r["nch"] * 128]
            blocks.append(np.tile(seg.reshape(-1, 16).T, (8, 1)))  # [128, nch*8]
            blocks.append(slot_bf[:, r["ch_base"] : r["ch_base"] + r["nch"]])
        gw = np.hstack(blocks)  # [128, totch*9]

        ed_full = dst[core_of == i] - i * SHARD
        deg = np.bincount(ed_full, minlength=SHARD)
        recip = np.ones(SHARD_PAD, np.float32)
        recip[perms[i]] = (1.0 / np.maximum(deg, 1)).astype(np.float32)
        # acc is initialized with bias/recip so the epilogue's single
        # activation (ot = acc * recip) yields agg*recip + bias -- no
        # separate DVE bias-add pass.
        acc_init = (1.0 / recip)[:, None] * np.asarray(bias, np.float32)[None, :]
        acc_init = np.ascontiguousarray(
            acc_init.reshape(NW, 128, D_OUT).transpose(1, 0, 2).reshape(128, NW * D_OUT)
        ).astype(np.float32)  # [128(slot), NW*128(f)]
        recip = np.ascontiguousarray(recip.reshape(NW, 128).T)  # [128, NW]

        featT = np.ascontiguousarray(feat[i * SHARD : (i + 1) * SHARD].T).astype(
            ml_dtypes.bfloat16
        )  # [256, 12500] bf16

        in_maps.append(
            {
                "featT": featT,
                "wT": wT,
                "bias_bc": bias_bc,
                "iota_bc": iota_bc,
                "recip": recip,
                "acc_init": acc_init,
                "gidx": np.ascontiguousarray(gw),
            }
        )
    return in_maps, {"runs": runs, "totch": totch, "C_wq": C_wq, "perms": perms}


def _build(sched):
    import os

    from concourse import bacc, mybir, tile

    max_runs = int(os.environ.get("K_MAX_RUNS", "9999"))
    skip_ag = os.environ.get("K_SKIP_AG", "0") == "1"
    skip_gather = os.environ.get("K_SKIP_GATHER", "0") == "1"
    oh_dtype_name = os.environ.get("K_OH_DT", "bfloat16")
    no_packet = os.environ.get("K_NO_PACKET", "0") == "1"

    runs = sched["runs"]
    totch = sched["totch"]
    f32 = mybir.dt.float32
    bf16 = mybir.dt.bfloat16
    i16 = mybir.dt.int16
    oh_dt = getattr(mybir.dt, oh_dtype_name)

    nc = bacc.Bacc(num_devices=N_CORES, num_swdge_queues=4, dynamic_dma_scratch_size=32768)
    featT = nc.dram_tensor("featT", [D_IN, SHARD], bf16, kind="ExternalInput")
    wT = nc.dram_tensor("wT", [D_IN, D_OUT], f32, kind="ExternalInput")
    bias_bc = nc.dram_tensor("bias_bc", [128, D_OUT], f32, kind="ExternalInput")
    iota_in = nc.dram_tensor("iota_bc", [128, 128], bf16, kind="ExternalInput")
    recip_in = nc.dram_tensor("recip", [128, NW], f32, kind="ExternalInput")
    acc_init_in = nc.dram_tensor("acc_init", [128, NW * D_OUT], f32, kind="ExternalInput")
    gidx_in = nc.dram_tensor("gidx", [128, totch * 9], i16, kind="ExternalInput")
    out = nc.dram_tensor("out", [SHARD_PAD, D_OUT], f32, kind="ExternalOutput")

    # AG slice q fires after the prologue tile containing row O[q+1]-1
    ag_after_tile = {(O[q + 1] + 127) // 128 - 1: q for q in range(NQ)}

    with tile.TileContext(nc) as tc:
        with (
            tc.tile_pool(name="dram", bufs=1, space="DRAM") as dram,
            tc.tile_pool(name="const", bufs=1) as constp,
            tc.tile_pool(name="ft", bufs=1) as ftp,
            tc.tile_pool(name="accp", bufs=1) as accp,
            tc.tile_pool(name="sb", bufs=2) as sb,
            tc.tile_pool(name="hbp", bufs=4) as hbp,
            tc.tile_pool(name="psA", bufs=2, space="PSUM") as psA,
            tc.tile_pool(name="psB", bufs=6, space="PSUM") as psB,
        ):
            h_loc = [
                dram.tile([O[q + 1] - O[q], D_OUT], bf16, name=f"h_loc{q}")
                for q in range(NQ)
            ]
            h_slice = [
                dram.tile(
                    [8 * (O[q + 1] - O[q]), D_OUT],
                    bf16,
                    addr_space="Shared",
                    name=f"h_slice{q}",
                )
                for q in range(NQ)
            ]

            # constants
            wt = constp.tile([128, 2, D_OUT], bf16)
            nc.gpsimd.dma_start(out=wt[:], in_=wT[:, :].rearrange("(a k) n -> k a n", k=128))
            biast = constp.tile([128, D_OUT], f32)
            nc.sync.dma_start(out=biast[:], in_=bias_bc[:, :])
            iotab = constp.tile([128, 128], bf16)
            nc.sync.dma_start(out=iotab[:], in_=iota_in[:, :])
            recip = constp.tile([128, NW], f32)
            nc.sync.dma_start(out=recip[:], in_=recip_in[:, :])

            acc = accp.tile([128, NW, D_OUT], f32)
            nc.sync.dma_start(
                out=acc[:], in_=acc_init_in[:, :].rearrange("p (w f) -> p w f", w=NW)
            )

            # ---- prologue: h = (feat @ w.T) bf16, sliced; AG per slice ----
            # featT slice tiles are padded out to 128-col tile boundaries so
            # every 128-row h tile's lhsT comes from ONE slice tile (psum
            # outputs must start at partition 0); only the h_loc store DMA
            # splits at slice boundaries.
            # featT slice loads are issued just-in-time at each slice start
            # (sync + scalar queues) so slice-0 h_loc writes are not queued
            # behind 24us of remaining featT loads on the serial Sync queue.
            ft = []
            ft_rng = []
            for q in range(NQ):
                plo = (O[q] // 128) * 128
                phi = min(((O[q + 1] + 127) // 128) * 128, SHARD)
                f = ftp.tile([128, 2, phi - plo], bf16, name=f"ft{q}")
                ft.append(f)
                ft_rng.append((plo, phi))
            Onp = np.asarray(O)
            loaded = set()
            for t in range(NT):
                lo = t * 128
                cnt = min(128, SHARD - lo)
                qt = int(np.searchsorted(Onp[1:], lo, side="right"))  # slice of tile
                if qt not in loaded:
                    loaded.add(qt)
                    plo, phi = ft_rng[qt]
                    nc.sync.dma_start(out=ft[qt][:, 0, :], in_=featT[0:128, plo:phi])
                    nc.scalar.dma_start(out=ft[qt][:, 1, :], in_=featT[128:256, plo:phi])
                ph = psA.tile([128, D_OUT], f32, space="PSUM")
                nc.tensor.matmul(
                    ph[:cnt, :],
                    lhsT=ft[qt][:, 0, lo - ft_rng[qt][0] : lo - ft_rng[qt][0] + cnt],
                    rhs=wt[:, 0, :],
                    start=True,
                    stop=False,
                )
                nc.tensor.matmul(
                    ph[:cnt, :],
                    lhsT=ft[qt][:, 1, lo - ft_rng[qt][0] : lo - ft_rng[qt][0] + cnt],
                    rhs=wt[:, 1, :],
                    start=False,
                    stop=True,
                )
                hb = hbp.tile([128, D_OUT], bf16)
                nc.scalar.activation(hb[:cnt, :], ph[:cnt, :], mybir.ActivationFunctionType.Copy)
                for q in range(NQ):
                    r0, r1 = max(lo, O[q]), min(lo + cnt, O[q + 1])
                    if r0 < r1:
                        nc.sync.dma_start(
                            out=h_loc[q][r0 - O[q] : r1 - O[q], :],
                            in_=hb[r0 - lo : r1 - lo, :],
                        )
                q = ag_after_tile.get(t)
                if q is not None:
                    if skip_ag:
                        nc.sync.dma_start(
                            out=h_slice[q][0 : O[q + 1] - O[q], :], in_=h_loc[q][:]
                        )
                    else:
                        nc.gpsimd.collective_compute(
                            "AllGather",
                            mybir.AluOpType.bypass,
                            replica_groups=[list(range(N_CORES))],
                            ins=[h_loc[q][:].opt()],
                            outs=[h_slice[q][:].opt()],
                        )

            # ---- main loop: bucket-major passes ----
            # one combined idx+slot stream load per pass (4 big DMAs on the
            # otherwise-idle Scalar HWDGE queue) -- many tiny per-run loads
            # head-of-line block the serial Sync queue.
            qrr = [0]
            pass_tiles = {}
            for q in range(NQ):
                pruns = [r for r in runs if r["q"] == q and r["nch"] > 0]
                c0 = pruns[0]["gs_off"]
                c1 = pruns[-1]["gs_off"] + pruns[-1]["nch"] * 9
                gsp = sb.tile([128, c1 - c0], i16, tag="gsp", bufs=4, name=f"gsp{q}")
                nc.scalar.dma_start(out=gsp[:], in_=gidx_in[:, c0:c1])
                pass_tiles[q] = (gsp, c0)
            for r in runs[:max_runs]:
                nch = r["nch"]
                if nch == 0:
                    continue
                q = r["q"]
                gsp, pc0 = pass_tiles[q]
                ro = r["gs_off"] - pc0
                gi = gsp[:, ro : ro + nch * 8]
                slt = gsp[:, ro + nch * 8 : ro + nch * 9].bitcast(bf16)
                ohb = sb.tile([128, nch, 128], oh_dt, tag="ohb", bufs=3)
                nc.vector.tensor_tensor(
                    out=ohb[:],
                    in0=slt[:, :, None].broadcast_to([128, nch, 128]),
                    in1=iotab[:, None, :].broadcast_to([128, nch, 128]),
                    op=mybir.AluOpType.is_equal,
                )
                msg = sb.tile([128, nch, D_OUT], bf16, tag="msg", bufs=6)
                if not skip_gather:
                    if no_packet:
                        # one unpacketed call per run: trades per-descriptor
                        # drain efficiency for ~half the Pool-engine call
                        # overhead (994ns fixed per SWDGE op)
                        nc.gpsimd.dma_gather(
                            msg[:, :, :],
                            h_slice[q][:],
                            gi[:, : nch * 8],
                            num_idxs=nch * 128,
                            num_idxs_reg=nch * 128,
                            elem_size=D_OUT,
                            queue_num=qrr[0] % 4,
                            single_packet=False,
                        )
                        qrr[0] += 1
                    else:
                        for s0 in range(0, nch, 8):
                            sc = min(8, nch - s0)
                            nc.gpsimd.dma_gather(
                                msg[:, s0 : s0 + sc, :],
                                h_slice[q][:],
                                gi[:, s0 * 8 : (s0 + sc) * 8],
                                num_idxs=sc * 128,
                                num_idxs_reg=sc * 128,
                                elem_size=D_OUT,
                                queue_num=qrr[0] % 4,
                                single_packet=True,
                            )
                            qrr[0] += 1
                ot = None
                if q == NQ - 1:
                    ot = sb.tile([128, len(r["ws"]), D_OUT], f32, tag="ot")
                for wi, w in enumerate(r["ws"]):
                    runs_w = [(woff, c) for (ww, woff, c) in r["wl"] if ww == w]
                    total_c = sum(c for _, c in runs_w)
                    if total_c == 0:
                        continue
                    pw = psB.tile([128, D_OUT], f32, space="PSUM", tag="pw")
                    done = 0
                    for (base, c) in runs_w:
                        for j in range(c):
                            nc.tensor.matmul(
                                pw[:, :],
                                lhsT=ohb[:, base + j, :],
                                rhs=msg[:, base + j, :],
                                start=(done == 0),
                                stop=(done == total_c - 1),
                            )
                            done += 1
                    nc.vector.tensor_add(acc[:, w, :], acc[:, w, :], pw[:, :])
                    if q == NQ - 1:
                        nc.scalar.activation(
                            ot[:, wi, :], acc[:, w, :], mybir.ActivationFunctionType.Copy,
                            scale=recip[:, w : w + 1],
                        )
                if q == NQ - 1:
                    nw_b = len(r["ws"])
                    w0 = r["ws"][0]
                    nc.sync.dma_start(
                        out=out[w0 * 128 : w0 * 128 + nw_b * 128, :].rearrange(
                            "(c p) f -> p c f", p=128
                        ),
                        in_=ot[:],
                    )

    nc.finalize()
    return nc


def _run(inputs, trace=False):
    from concourse.bass_utils import run_bass_kernel_spmd

    key = "k"
    in_maps, sched = _preprocess(
        inputs["feat"], inputs["w_neigh"], inputs["bias"], inputs["src"], inputs["dst"]
    )
    if key not in _cache:
        _cache[key] = _build(sched)
    nc = _cache[key]
    res = run_bass_kernel_spmd(nc, in_maps, core_ids=list(range(N_CORES)), trace=trace)
    outs = [res.results[i]["out"][sched["perms"][i]] for i in range(N_CORES)]
    full = np.concatenate(outs, axis=0)
    return full, res


def kernel(**inputs):
    full, _ = _run(inputs, trace=False)
    return full
